# revision 1
# baseline (speedup 1.0000x reference)
"""MoE routing + grouped SwiGLU MLP for Trainium2 (8 NeuronCores).

Strategy (expert-parallel, E == n_cores == 8):
  * Host: router matmul, softmax, top-2 selection, token->expert
    gather, weight repacking, aux losses (all cheap: ~25 MFLOP of the
    ~87 GFLOP total).
  * Device, per core e: dense SwiGLU for expert e over its
    capacity-padded token set (C tokens, C = max group size rounded to
    a multiple of 128):
        h1T = w1.T @ x    [I, C]   (fp32r matmuls, fp32 PSUM accum)
        h2T = w2.T @ x    [I, C]
        gT  = silu(h1T) * h2T      (silu = a*sigmoid(a); the ACT Silu
                                    LUT faults on this HW, sigmoid works)
        yT  = w3.T @ gT   [D, C]
    Activations are the stationary matmul operand; weights stream from
    HBM once (85 MB/core) in host-packed, per-partition-contiguous
    layouts.  float32r keeps fp32 storage with ~1 cycle/row PE rate;
    measured end-to-end error ~3e-4.
  * Host: scatter-add y back per token with the normalized top-2
    weights.

Returns (out, expert_loads, lbl, rzl) exactly like the reference.
"""
import sys
for _p in ('/opt/trn_rl_repo',):
    if _p not in sys.path:
        sys.path.insert(0, _p)

import numpy as np
import bass_rust
import concourse.bass as bass
import concourse.tile as tile
from concourse import mybir
from concourse.vector_clock import ScopedClock

D = 1536
I = 4608
E = 8
K = 2
N_CORES = 8
KT = D // 128   # 12 k-tiles  (x.T @ w contraction)
IT = I // 128   # 36 i-tiles
DT = D // 128   # 12 d-tiles

F32 = mybir.dt.float32
F32R = mybir.dt.float32r

# ---------------------------------------------------------------------------
# TileContext workaround: this walrus build accepts at most ONE sync-wait
# command per instruction (CTRL / S3_LW structs).  Split excess waits onto
# same-engine nops emitted immediately before the instruction, and do the
# same for the kernel-tail drain.
_MAXW = 1


class PatchedTileContext(tile.TileContext):
    def _add_instruction(self, inst):
        si = inst.sync_info
        waits = list(si.on_wait) if si is not None and si.on_wait else []
        if len(waits) > _MAXW:
            nc = self.nc
            extra, keep = waits[:-_MAXW], waits[-_MAXW:]
            for j in range(0, len(extra), _MAXW):
                nop = bass_rust.InstNoOp(
                    name=nc.get_next_instruction_name(),
                    engine=inst.engine, ins=[], outs=[],
                )
                nop.sync_info = bass_rust.SyncInfo(
                    on_wait=extra[j:j + _MAXW], on_update=[])
                super()._add_instruction(nop)
            inst.sync_info = bass_rust.SyncInfo(
                on_wait=keep,
                on_update=list(si.on_update) if si.on_update else [],
            )
        super()._add_instruction(inst)

    def _drain_and_barrier(self, tick_clock, wait_clock):
        nc = self.nc
        carrier = nc.sync.nop(nofuse=True)
        wait_clock.add_sem_waits(
            carrier.ins, ScopedClock({None: tick_clock.global_clock}))
        si = carrier.ins.sync_info
        waits = list(si.on_wait) if si is not None and si.on_wait else []
        if len(waits) > _MAXW:
            carrier.ins.sync_info = bass_rust.SyncInfo(
                on_wait=waits[:_MAXW],
                on_update=list(si.on_update) if si.on_update else [],
            )
            rest = waits[_MAXW:]
            for j in range(0, len(rest), _MAXW):
                n = nc.sync.nop(nofuse=True)
                n.ins.sync_info = bass_rust.SyncInfo(
                    on_wait=rest[j:j + _MAXW], on_update=[])
        nc.sync.drain()
        nc.all_engine_barrier()
        assert self.sems is not None
        popped = nc._tile_sem_poison_stack.pop()
        assert popped is self._sem_poison
        nc.clear_and_free_semaphores(list(self.sems.allocated().values()))
        nc.all_engine_barrier()


# ---------------------------------------------------------------------------
# Device program


def _chunks_for(C):
    n = -(-C // 512)
    sizes, rem = [], C
    for j in range(n):
        s = min(512, ((rem // (n - j)) + 127) // 128 * 128)
        sizes.append(s)
        rem -= s
    assert rem == 0 and all(128 <= s <= 512 for s in sizes)
    offs = np.cumsum([0] + sizes[:-1]).tolist()
    return list(zip(offs, sizes))


def _build_moe_nc(C):
    """One expert's SwiGLU over C capacity-padded tokens (SPMD, 8 cores)."""
    assert C % 128 == 0 and C >= 256
    CH = _chunks_for(C)
    nc = bass.Bass("TRN2", target_bir_lowering=False, debug=False)
    XT = nc.dram_tensor("xt", [KT, 128, C], F32R, kind="ExternalInput").ap()
    W12P = nc.dram_tensor(
        "w12p", [IT, 128, 2, KT, 128], F32R, kind="ExternalInput").ap()
    W3P = nc.dram_tensor(
        "w3p", [DT, 128, IT, 128], F32R, kind="ExternalInput").ap()
    YT = nc.dram_tensor("yt", [DT, 128, C], F32, kind="ExternalOutput").ap()
    sig = mybir.ActivationFunctionType.Sigmoid

    with PatchedTileContext(nc) as tc:
        with (
            tc.tile_pool(name="xp", bufs=1) as xp,
            tc.tile_pool(name="gp", bufs=1) as gp,
            tc.tile_pool(name="wp", bufs=3) as wp,
            tc.tile_pool(name="w3pool", bufs=3) as w3pool,
            tc.tile_pool(name="sp", bufs=3) as spool,
            tc.tile_pool(name="op", bufs=3) as opool,
            tc.tile_pool(name="ps", bufs=6, space="PSUM") as psp,
        ):
            xt = xp.tile([128, KT, C], F32R, tag="xt", name="xt_t")
            nc.sync.dma_start(xt[:], XT.rearrange("k p c -> p k c"))
            gt = gp.tile([128, IT, C], F32R, tag="gt", name="gt_t")
            for i in range(IT):
                w12i = wp.tile([128, 2, KT, 128], F32R, tag="w12",
                               name="w12i")
                nc.sync.dma_start(w12i[:], W12P[i])
                for (off, sz) in CH:
                    ps1 = psp.tile([128, sz], F32, tag="ps", name="ps1")
                    ps2 = psp.tile([128, sz], F32, tag="ps", name="ps2")
                    for k in range(KT):
                        nc.tensor.matmul(
                            ps1[:], w12i[:, 0, k], xt[:, k, off:off + sz],
                            start=(k == 0), stop=(k == KT - 1))
                    for k in range(KT):
                        nc.tensor.matmul(
                            ps2[:], w12i[:, 1, k], xt[:, k, off:off + sz],
                            start=(k == 0), stop=(k == KT - 1))
                    s1 = spool.tile([128, sz], F32, tag="s1", name="s1")
                    nc.scalar.activation(s1[:], ps1[:], sig)
                    t = spool.tile([128, sz], F32, tag="t", name="t")
                    nc.vector.tensor_mul(t[:], s1[:], ps1[:])
                    nc.vector.tensor_mul(gt[:, i, off:off + sz], t[:], ps2[:])
            for d in range(DT):
                w3d = w3pool.tile([128, IT, 128], F32R, tag="w3", name="w3d")
                nc.sync.dma_start(w3d[:], W3P[d])
                for (off, sz) in CH:
                    ps = psp.tile([128, sz], F32, tag="ps", name="ps3")
                    for kp in range(IT):
                        nc.tensor.matmul(
                            ps[:], w3d[:, kp], gt[:, kp, off:off + sz],
                            start=(kp == 0), stop=(kp == IT - 1))
                    o = opool.tile([128, sz], F32, tag="o", name="o")
                    nc.vector.tensor_copy(o[:], ps[:])
                    nc.sync.dma_start(YT[d, :, off:off + sz], o[:])
    return nc


_NC_CACHE = {}


def _get_nc(C):
    if C not in _NC_CACHE:
        _NC_CACHE[C] = _build_moe_nc(C)
    return _NC_CACHE[C]


# ---------------------------------------------------------------------------
# Host-side packing / routing


def _pack_weights(w1, w2, w3):
    w1p = w1.reshape(KT, 128, IT, 128).transpose(2, 1, 0, 3)
    w2p = w2.reshape(KT, 128, IT, 128).transpose(2, 1, 0, 3)
    w12p = np.ascontiguousarray(np.stack([w1p, w2p], axis=2))
    w3p = np.ascontiguousarray(
        w3.reshape(IT, 128, DT, 128).transpose(2, 1, 0, 3))
    return w12p, w3p


def _pack_x(x_rows, C):
    xp = np.zeros((C, D), dtype=np.float32)
    xp[:x_rows.shape[0]] = x_rows
    return np.ascontiguousarray(xp.T.reshape(KT, 128, C))


def _route_host(x_flat, router_w):
    lg = x_flat.astype(np.float64) @ router_w.astype(np.float64)  # [T, E]
    lmax = lg.max(axis=-1, keepdims=True)
    ex = np.exp(lg - lmax)
    probs = ex / ex.sum(axis=-1, keepdims=True)
    t_idx = np.arange(lg.shape[0])
    e1 = np.argmax(lg, axis=-1)
    lg2 = lg.copy()
    lg2[t_idx, e1] = -np.inf
    e2 = np.argmax(lg2, axis=-1)
    p1 = probs[t_idx, e1]
    p2 = probs[t_idx, e2]
    s = p1 + p2
    return e1, e2, p1 / s, p2 / s, probs, lg


def kernel(x, router_w, w1, w2, w3):
    from concourse.bass_utils import run_bass_kernel_spmd

    x = np.asarray(x, dtype=np.float32)
    router_w = np.asarray(router_w, dtype=np.float32)
    w1 = np.asarray(w1, dtype=np.float32)
    w2 = np.asarray(w2, dtype=np.float32)
    w3 = np.asarray(w3, dtype=np.float32)

    B, S, _ = x.shape
    T = B * S
    x_flat = np.ascontiguousarray(x.reshape(T, D))

    e1, e2, wt1, wt2, probs, lg = _route_host(x_flat, router_w)
    flat_idx = np.stack([e1, e2], 1).reshape(-1)
    gs = np.bincount(flat_idx, minlength=E)

    C = max(256, int(-(-gs.max() // 128) * 128))
    nc = _get_nc(C)

    tok_lists, wt_lists, in_maps = [], [], []
    for e in range(E):
        m1 = e1 == e
        m2 = e2 == e
        toks = np.concatenate([np.where(m1)[0], np.where(m2)[0]])
        wts = np.concatenate([wt1[m1], wt2[m2]])
        tok_lists.append(toks)
        wt_lists.append(wts)
        w12p, w3p = _pack_weights(w1[e], w2[e], w3[e])
        in_maps.append({
            "xt": _pack_x(x_flat[toks], C),
            "w12p": w12p, "w3p": w3p,
        })

    res = run_bass_kernel_spmd(nc, in_maps, core_ids=list(range(N_CORES)))

    out_flat = np.zeros((T, D), dtype=np.float64)
    for e in range(E):
        n = len(tok_lists[e])
        y = res.results[e]["yt"].reshape(D, C).T[:n]
        out_flat[tok_lists[e]] += wt_lists[e][:, None] * y.astype(np.float64)
    out = out_flat.astype(np.float32).reshape(B, S, D)

    gs_f = gs.astype(np.float64)
    expert_loads = (gs_f / gs_f.sum()).astype(np.float32)
    f = expert_loads.astype(np.float64) * (E / K)
    p = probs.mean(axis=0)
    lbl = np.float32(0.01 * np.sum(f * p))
    lmax = lg.max(-1, keepdims=True)
    z = np.log(np.exp(lg - lmax).sum(-1)) + lmax[:, 0]
    rzl = np.float32(0.001 * np.mean(z ** 2))

    return out, expert_loads, lbl, rzl
